# revision 17
# baseline (speedup 1.0000x reference)
"""Trainium2 Bass kernel for nn_MFFNet8 (MFFNet8 skeleton-action model).

Self-contained: hardcodes shapes; shards batch N across 8 cores (4 samples =
8 person-streams per core); runs the full network per stream SBUF-resident.

Layout: activations channel-major (C, T*V) per stream, (t,v) row-major free.
GCN einsum + attention S-mix run in PE-transposed space on 126-col blocks
(7 t x 18 v): transposed activations are produced directly as matmuls with
the activation block as the stationary operand (hT = x_blk.T @ W), mixed with
block-diagonal adjacency, bias added via a rank-4 matmul, residual accumulated
in transposed space, then one back-transpose + fused ReLU epilogue.
All batchnorms are folded into conv weights/biases on the host.
Compute dtype bf16 (fp32 accumulate); the 3-channel stem stays fp32.
"""
import os
import numpy as np
import ml_dtypes
from contextlib import ExitStack

import concourse.bass as bass
import concourse.tile as tile
from concourse import mybir
from concourse.bass_utils import run_bass_kernel_spmd

N, M, C, T, V = 32, 2, 3, 300, 18
NUM_CLASS = 400
EPS = 1e-5
NCORES = 8
NLOC = N // NCORES
NSTREAM = NLOC * M
S300, S150, S50 = 300 * V, 150 * V, 50 * V
CHUNK = 504

F32 = mybir.dt.float32
USE_BF16 = os.environ.get("USE_BF16", "1") == "1"
DT = mybir.dt.bfloat16 if USE_BF16 else F32
NPDT = ml_dtypes.bfloat16 if USE_BF16 else np.float32
AF = mybir.ActivationFunctionType
ALU = mybir.AluOpType
AX = mybir.AxisListType


# ------------------------------------------------------------------
# Host-side weight preparation
# ------------------------------------------------------------------

def _np32(x):
    return np.asarray(x, dtype=np.float32)


def _bn_sb(p):
    s = _np32(p['g']) / np.sqrt(_np32(p['v']) + EPS)
    b = _np32(p['b']) - _np32(p['m']) * s
    return s, b


def prep_weights(params):
    W = {}
    p = params

    s_d, b_d = _bn_sb(p['data_bn'])
    W['dbn_s'] = s_d.reshape(V, C).T.copy()
    W['dbn_b'] = b_d.reshape(V, C).T.copy()

    def conv_lhsT(cp, rows=None):
        w = _np32(cp['w'])[:, :, 0, 0]
        if rows is not None:
            w = w[rows]
        return w.T.copy()

    W['cs1_w'] = conv_lhsT(p['conv_shift_1'])
    W['cs1_b'] = _np32(p['conv_shift_1']['b'])

    def gcn_consts(gp, A, tag):
        A = _np32(A)
        s, b = _bn_sb(gp['bn'])
        w = _np32(gp['w'])[:, :, 0, 0]
        co = w.shape[0] // 3
        for k in range(3):
            wk = w[k * co:(k + 1) * co] * s[:, None]
            W[f'{tag}_w{k}'] = wk.T.copy()
            bd = np.zeros((126, 126), np.float32)
            for j in range(7):
                bd[j * 18:(j + 1) * 18, j * 18:(j + 1) * 18] = A[k]
            W[f'{tag}_bd{k}'] = bd
        alph = A.sum(axis=1)
        lb = np.zeros((4, 126), np.float32)
        for j in range(7):
            lb[0:3, j * 18:(j + 1) * 18] = alph
            lb[3, j * 18:(j + 1) * 18] = 1.0
        rb = np.zeros((4, co), np.float32)
        for k in range(3):
            rb[k] = s * _np32(gp['b'])[k * co:(k + 1) * co]
        rb[3] = b
        if gp['res'] is not None:
            sr, br = _bn_sb(gp['res']['bn'])
            wr = _np32(gp['res']['w'])[:, :, 0, 0] * sr[:, None]
            W[f'{tag}_rw'] = wr.T.copy()
            # fold res-conv bias into the all-ones rank-1 bias term
            rb[3] = rb[3] + sr * _np32(gp['res']['b']) + br
        W[f'{tag}_blhsT'] = lb
        W[f'{tag}_brhs'] = rb

    gcn_consts(p['gcn_shift_1'], p['A_shift_1'], 'g1')
    gcn_consts(p['gcn_shift_2'], p['A_shift_2'], 'g2')
    for i in range(4):
        gcn_consts(p['gcn_list'][i], p['As'][i], f'gl{i}')

    def tcn_consts(tp, tag):
        s1, b1 = _bn_sb(tp['bn1'])
        s2, b2 = _bn_sb(tp['bn2'])
        w = _np32(tp['w'])[:, :, :, 0]
        for tap in range(3):
            W[f'{tag}_w{tap}'] = (w[:, :, tap] * s1[None, :] * s2[:, None]).T.copy()
        W[f'{tag}_pad'] = (-b1 / s1).astype(np.float32).reshape(-1, 1)
        W[f'{tag}_b'] = (s2 * (_np32(tp['b']) + w.sum(axis=2) @ b1) + b2).reshape(-1, 1)

    tcn_consts(p['tcn_shift'], 'ts')
    tcn_consts(p['tcn_motion'], 'tm')
    tcn_consts(p['tcn_pos1'], 'tp1')
    tcn_consts(p['tcn_pos2'], 'tp2')
    for i in range(4):
        tcn_consts(p['tcn_list'][i], f'tl{i}')

    W['cs2_w'] = conv_lhsT(p['conv_shift_2'], rows=[0, 1])
    W['cs2_b'] = _np32(p['conv_shift_2']['b'])[:2]
    fw = conv_lhsT(p['conv_fusion'])
    W['fu_w0'], W['fu_w1'] = fw[:64].copy(), fw[64:].copy()
    W['fu_b'] = _np32(p['conv_fusion']['b'])

    # gcn_list inputs are concat(64, 64): split conv/res weights at row 64
    for i in range(4):
        for k in range(3):
            wk = W.pop(f'gl{i}_w{k}')
            W[f'gl{i}_w{k}_a'], W[f'gl{i}_w{k}_b'] = wk[:64].copy(), wk[64:].copy()
        wr = W.pop(f'gl{i}_rw')
        W[f'gl{i}_rw_a'], W[f'gl{i}_rw_b'] = wr[:64].copy(), wr[64:].copy()

    for tag, gp in (('ga1', p['gather1']), ('ga2', p['gather2'])):
        for i in range(3):
            W[f'{tag}_aw{i}'] = _np32(gp['a_w'][i])[:, :, 0, 0].T.copy()
            W[f'{tag}_ab{i}'] = _np32(gp['a_b'][i])
            W[f'{tag}_dw{i}'] = _np32(gp['d_w'][i])[:, :, 0, 0].T.copy()
        W[f'{tag}_db'] = sum(_np32(gp['d_b'][i]) for i in range(3))

    W['fcn_w'] = _np32(p['fcn']['w'])[:, :, 0, 0].T.copy()
    W['fcn_b'] = np.tile(_np32(p['fcn']['b'])[None, :], (NLOC, 1)).copy()
    W['ident'] = np.eye(128, dtype=np.float32)
    erep = np.zeros((18, 126), np.float32)
    for j in range(7):
        for v in range(18):
            erep[v, j * 18 + v] = 1.0
    W['erep'] = erep
    bmask = np.zeros((126, 126), np.float32)
    for j in range(7):
        bmask[j * 18:(j + 1) * 18, j * 18:(j + 1) * 18] = 1.0
    W['bdmask'] = bmask

    out = {}

    F32K = {'dbn_s', 'dbn_b', 'fcn_b', 'cs1_w', 'cs1_b', 'cs2_b', 'fu_b',
            'ts_b', 'tm_b', 'tp1_b', 'tp2_b', 'tl0_b', 'tl1_b', 'tl2_b',
            'tl3_b'}

    def is_f32(k):
        return (k in F32K or k.endswith('_pad')
                or k.startswith(('ga1_ab', 'ga2_ab', 'ga1_db', 'ga2_db')))

    for k, v in W.items():
        v = np.asarray(v, np.float32)
        if v.ndim == 1:
            v = v.reshape(-1, 1)
        dt = np.float32 if is_f32(k) else NPDT
        v = v.astype(dt)
        if v.shape[0] > 128:
            assert v.shape[0] == 256, k
            out[k + '_p0'] = np.ascontiguousarray(v[:128])
            out[k + '_p1'] = np.ascontiguousarray(v[128:])
        else:
            out[k] = np.ascontiguousarray(v)
    return out


def wmeta(W):
    np2dt = {np.dtype(np.float32): F32,
             np.dtype(ml_dtypes.bfloat16): mybir.dt.bfloat16}
    return {k: (v.shape, np2dt[v.dtype]) for k, v in W.items()}


def _chunks(S, sz=CHUNK):
    return [(c0, min(c0 + sz, S)) for c0 in range(0, S, sz)]


# ------------------------------------------------------------------
# Kernel builder
# ------------------------------------------------------------------

class KB:
    def __init__(self, nc, tc):
        self.nc = nc
        self.tc = tc
        self.w = {}

    def load_weights(self, ctx, dram, meta):
        nc = self.nc
        pool = ctx.enter_context(self.tc.tile_pool(name="wpool", bufs=1))
        for name, (shp, dt) in meta.items():
            t = pool.tile(list(shp), dt, tag=f"w_{name}", name=f"w_{name}")
            nc.sync.dma_start(out=t[:], in_=dram[name][:])
            self.w[name] = t

    # ---- primitives ----

    def mm(self, out, lhsT, rhs, **kw):
        self.nc.tensor.matmul(out, lhsT, rhs, **kw)

    def copy(self, dst, src, use_act=False):
        if use_act:
            self.nc.scalar.copy(dst, src)
        else:
            self.nc.vector.tensor_copy(dst, src)

    def conv(self, dst, srcs, lhsTs, S, bias=None, relu=False, dst_off=0,
             add_into=False):
        nc = self.nc
        co = lhsTs[0].shape[1]
        for c0, c1 in _chunks(S):
            wd = c1 - c0
            ps = self.pmm.tile([128, CHUNK], F32, tag="mm")
            for ki, (src, lh) in enumerate(zip(srcs, lhsTs)):
                self.mm(ps[:co, :wd], lh, src[:, c0:c1],
                        start=(ki == 0), stop=(ki == len(srcs) - 1))
            dsl = dst[:, dst_off + c0:dst_off + c1]
            if add_into:
                nc.vector.scalar_tensor_tensor(dsl, ps[:co, :wd],
                                               bias if bias is not None else 0.0,
                                               dsl, op0=ALU.add, op1=ALU.add)
            elif relu or bias is not None:
                nc.scalar.activation(dsl, ps[:co, :wd],
                                     AF.Relu if relu else AF.Identity,
                                     bias=bias if bias is not None else 0.0)
            else:
                nc.vector.tensor_copy(dsl, ps[:co, :wd])

    def tcn(self, dst, pb, wtag, Tin, stride, dst_off=0):
        nc = self.nc
        Tout = Tin // stride
        Sout = Tout * V
        pb3 = pb.rearrange("c (t v) -> c t v", v=V)
        co = self.w[f'{wtag}_w0'].shape[1]
        for c0, c1 in _chunks(Sout):
            wd = c1 - c0
            ta, tb = c0 // V, c1 // V
            ps = self.pmm.tile([128, CHUNK], F32, tag="mm")
            for tap in range(3):
                rhs = pb3[:, ta * stride + tap:(tb - 1) * stride + tap + 1:stride, :]
                self.mm(ps[:co, :wd], self.w[f'{wtag}_w{tap}'][:], rhs,
                        start=(tap == 0), stop=(tap == 2))
            nc.scalar.activation(dst[:, dst_off + c0:dst_off + c1], ps[:co, :wd],
                                 AF.Relu, bias=self.w[f'{wtag}_b'][:, 0:1])

    def tcn_pool(self, dst_pool, pb, wtag, Tin, factor):
        """tcn (stride 1) + immediate max-pool over t windows -> dst_pool."""
        nc = self.nc
        Sout = Tin * V
        pb3 = pb.rearrange("c (t v) -> c t v", v=V)
        co = self.w[f'{wtag}_w0'].shape[1]
        csz = 24 * V
        assert (24 % factor) == 0 and ((Tin % 24) % factor) == 0
        with ExitStack() as tctx:
            tp = tctx.enter_context(self.tc.tile_pool(name=f"tpool_{wtag}", bufs=2))
            for c0 in range(0, Sout, csz):
                c1 = min(c0 + csz, Sout)
                wd = c1 - c0
                ta, tb = c0 // V, c1 // V
                ps = self.pmm.tile([128, CHUNK], F32, tag="mm")
                for tap in range(3):
                    rhs = pb3[:, ta + tap:tb + tap, :]
                    self.mm(ps[:co, :wd], self.w[f'{wtag}_w{tap}'][:], rhs,
                            start=(tap == 0), stop=(tap == 2))
                tmp = tp.tile([128, csz], DT, tag="ptmp", name="ptmp")
                nc.scalar.activation(tmp[:co, :wd], ps[:co, :wd], AF.Relu,
                                     bias=self.w[f'{wtag}_b'][:, 0:1])
                nt2 = (tb - ta) // factor
                vin = bass.AP(tensor=tmp.tensor, offset=tmp.offset,
                              ap=[list(tmp.ap[0]), [factor * V, nt2], [1, V],
                                  [V, factor]])
                dsl = dst_pool[:co, (ta // factor) * V:(tb // factor) * V]
                nc.vector.tensor_reduce(dsl.rearrange("c (t v) -> c t v", v=V),
                                        vin[:co], axis=AX.X, op=ALU.max)

    def fill_pad(self, pb, wtag, Tin):
        nc = self.nc
        S = (Tin + 2) * V
        for off in (0, S - V):
            sl = pb[:, off:off + V]
            nc.vector.memset(sl, 0.0)
            nc.vector.tensor_scalar_add(sl, sl, self.w[f'{wtag}_pad'][:, 0:1])

    def transpose_to(self, src_ap, rows, cols):
        """PE-transpose src (rows<=128, cols<=128) -> psum (cols, rows)."""
        ps = self.ptp.tile([128, 256], src_ap.dtype, tag="tp",
                           name="tp_ps")
        self.nc.tensor.matmul(ps[:cols, :rows], src_ap,
                              self.w['ident'][:rows, :rows],
                              is_transpose=True, start=True, stop=True)
        return ps

    def gcn(self, dst, x_srcs, tag, Tt, res='conv'):
        """dst = relu(mix(conv(x)) + rank4bias + res).

        Per 126-block: hT_k = x_blk.T @ Wk (x_blk stationary); yT accumulates
        blockdiag-A mixes + rank-4 bias + residual; one back-transpose; fused
        ReLU epilogue. dst: AP (co<=128, S) or list of 2 tiles for co=256.
        """
        nc = self.nc
        S = Tt * V
        w0 = self.w.get(f'{tag}_w0')
        if w0 is None:
            w0 = self.w[f'{tag}_w0_a']
        co = w0.shape[1]
        nct = (co + 127) // 128

        def wparts(base):
            if len(x_srcs) == 1:
                return [self.w[base]]
            return [self.w[base + '_a'], self.w[base + '_b']]

        with ExitStack() as gctx:
            sml = gctx.enter_context(self.tc.tile_pool(name=f"g_{tag}", bufs=3))
            for b0 in range(0, S, 126):
                b1 = min(b0 + 126, S)
                blk = b1 - b0
                hTs = []
                for k in range(3):
                    ps = self.ptp.tile([128, 256], F32, tag="tp")
                    parts = wparts(f'{tag}_w{k}')
                    for ki, (src, lh) in enumerate(zip(x_srcs, parts)):
                        self.mm(ps[:blk, :co], src[:, b0:b1], lh[:],
                                start=(ki == 0), stop=(ki == len(parts) - 1))
                    sb = sml.tile([126, 256], DT, tag=f"hT{k}", name=f"hT{k}")
                    self.copy(sb[:blk, :co], ps[:blk, :co], use_act=(k == 1))
                    hTs.append(sb)
                yT = self.ptp.tile([128, 256], F32, tag="tp")
                for k in range(3):
                    self.mm(yT[:blk, :co], self.w[f'{tag}_bd{k}'][:blk, :blk],
                            hTs[k][:blk, :co], start=(k == 0), stop=False)
                self.mm(yT[:blk, :co], self.w[f'{tag}_blhsT'][:, :blk],
                        self.w[f'{tag}_brhs'][:, :co], start=False, stop=False)
                if isinstance(res, str):
                    parts = wparts(f'{tag}_rw')
                    for ki, (src, lh) in enumerate(zip(x_srcs, parts)):
                        self.mm(yT[:blk, :co], src[:, b0:b1], lh[:],
                                start=False, stop=(ki == len(parts) - 1))
                else:
                    self.mm(yT[:blk, :co], res[:, b0:b1],
                            self.w['ident'][:res.shape[0], :co],
                            start=False, stop=True)
                ysb = sml.tile([126, 256], DT, tag="yT", name="yT")
                self.copy(ysb[:blk, :co], yT[:blk, :co])
                for ct in range(nct):
                    r0, r1 = ct * 128, min(co, ct * 128 + 128)
                    yps = self.transpose_to(ysb[:blk, r0:r1], blk, r1 - r0)
                    dsl = (dst[ct][:, b0:b1] if isinstance(dst, list)
                           else dst[:, b0:b1])
                    nc.scalar.activation(dsl, yps[:r1 - r0, :blk], AF.Relu)

    def gather(self, dst, x_tiles, tag, Tt):
        """Attention block. x_tiles/dst: 2 tiles (128, S)."""
        nc = self.nc
        S = Tt * V
        w = self.w
        inv = 1.0 / (128 * Tt)
        with ExitStack() as gctx:
            big = gctx.enter_context(self.tc.tile_pool(name=f"gab_{tag}", bufs=1))
            sml = gctx.enter_context(self.tc.tile_pool(name=f"gas_{tag}", bufs=3))

            mp = big.tile([128, max(Tt, 1), 3, 32], DT, tag="mpack")
            nc.vector.memset(mp[:, :, :, V:32], 0.0)
            for i in range(3):
                for c0, c1 in _chunks(S):
                    wd = c1 - c0
                    ta, tb = c0 // V, c1 // V
                    ps = self.pmm.tile([128, CHUNK], F32, tag="mm")
                    for ki in range(2):
                        self.mm(ps[:, :wd], w[f'{tag}_aw{i}_p{ki}'][:],
                                x_tiles[ki][:, c0:c1], start=(ki == 0), stop=(ki == 1))
                    nc.scalar.activation(mp[:, ta:tb, i, 0:V],
                                         ps[:, :wd].rearrange("p (t v) -> p t v", v=V),
                                         AF.Identity, bias=w[f'{tag}_ab{i}'][:, 0:1])
            # Gram: lhsT padded to 3x32 groups so G_i lands 32-aligned
            gps = self.ptp.tile([128, 256], F32, tag="tp")
            for t in range(Tt):
                self.mm(gps[:96, :3 * V], mp[:, t, :, :], mp[:, t, :, 0:V],
                        start=(t == 0), stop=(t == Tt - 1))
            Sd = []
            for i in range(3):
                g = sml.tile([V, V], F32, tag="gsm")
                nc.vector.tensor_copy(g[:], gps[32 * i:32 * i + V, i * V:(i + 1) * V])
                mx = sml.tile([V, 1], F32, tag="gmx")
                nc.vector.tensor_reduce(mx[:], g[:], axis=AX.X, op=ALU.max)
                mxs = sml.tile([V, 1], F32, tag="gmxs")
                nc.scalar.activation(mxs[:], mx[:], AF.Copy, scale=-inv)
                e = sml.tile([V, V], F32, tag="gexp")
                nc.scalar.activation(e[:], g[:], AF.Exp, bias=mxs[:, 0:1], scale=inv)
                sm = sml.tile([V, 1], F32, tag="gsum")
                nc.vector.tensor_reduce(sm[:], e[:], axis=AX.X, op=ALU.add)
                rc = sml.tile([V, 1], F32, tag="grc")
                nc.vector.reciprocal(rc[:], sm[:])
                st = sml.tile([V, V], DT, tag="gst")
                nc.scalar.activation(st[:], e[:], AF.Copy, scale=rc[:, 0:1])
                sps = self.transpose_to(st[:], V, V)
                sd = sml.tile([V, V], DT, tag=f"gsd{i}", name=f"gsd{i}")
                nc.vector.tensor_copy(sd[:], sps[:V, :V])
                Sd.append(sd)
            if Tt > 1:
                bds = []
                for i in range(3):
                    pp = self.ptp.tile([128, 256], F32, tag="tp")
                    self.mm(pp[:126, :V], w['erep'][:], Sd[i][:], start=True, stop=True)
                    srep = sml.tile([126, V], DT, tag="srep", name="srep")
                    nc.vector.tensor_copy(srep[:], pp[:126, :V])
                    bd = big.tile([126, 126], DT, tag=f"bds{i}", name=f"bds{i}")
                    s_b = bass.AP(tensor=srep.tensor, offset=srep.offset,
                                  ap=[list(srep.ap[0]), [0, 7], [1, V]])
                    nc.vector.tensor_tensor(
                        bd.rearrange("p (j2 w) -> p j2 w", j2=7),
                        w['bdmask'].rearrange("p (j2 w) -> p j2 w", j2=7),
                        s_b, op=ALU.mult)
                    bds.append(bd)
            else:
                bds = Sd
            # mix: per block, uT_i = x_blk.T @ Dw_i; y = sum_i uT_i.T @ BD_i
            for b0 in range(0, S, 126):
                b1 = min(b0 + 126, S)
                blk = b1 - b0
                for ct in range(2):
                    uTs = []
                    for i in range(3):
                        ps = self.ptp.tile([128, 256], F32, tag="tp")
                        for ki in range(2):
                            lh = w[f'{tag}_dw{i}_p{ki}'][:, ct * 128:(ct + 1) * 128]
                            self.mm(ps[:blk, :128], x_tiles[ki][:, b0:b1], lh,
                                    start=(ki == 0), stop=(ki == 1))
                        sb = sml.tile([126, 128], DT, tag=f"uT{i}", name=f"uT{i}")
                        self.copy(sb[:blk, :], ps[:blk, :128], use_act=(i == 1))
                        uTs.append(sb)
                    yps = self.ptp.tile([128, 256], F32, tag="tp")
                    for i in range(3):
                        self.mm(yps[:128, :blk], uTs[i][:blk, :], bds[i][:blk, :blk],
                                start=(i == 0), stop=(i == 2))
                    nc.scalar.activation(dst[ct][:, b0:b1], yps[:128, :blk],
                                         AF.Identity, bias=w[f'{tag}_db_p{ct}'][:, 0:1])


def pool_T_max(nc, dst, src_ap, Tin, factor):
    v = src_ap.rearrange("c (t2 k v) -> c t2 v k", k=factor, v=V)
    nc.vector.tensor_reduce(dst.rearrange("c (t v) -> c t v", v=V), v,
                            axis=AX.X, op=ALU.max)


def split_multiwaits(nc, limit=1):
    nsplit = 0
    for f in nc.m.functions:
        for blk in f.blocks:
            insts = list(blk.instructions)
            out = []
            changed = False
            for inst in insts:
                si = inst.sync_info
                if si is not None and si.on_wait is not None and len(si.on_wait) > limit:
                    waits = list(si.on_wait)
                    head, tail = waits[:-limit], waits[-limit:]
                    for j, ww in enumerate(head):
                        nop = mybir.InstNoOp(name=f"{inst.name}-wsplit{j}", ins=[], outs=[])
                        nop.engine = inst.engine
                        nop.sync_info = mybir.SyncInfo(on_update=[], on_wait=[ww])
                        out.append(nop)
                    inst.sync_info = mybir.SyncInfo(on_update=list(si.on_update or []),
                                                   on_wait=tail)
                    nsplit += 1
                    changed = True
                out.append(inst)
            if changed:
                blk.instructions = out
    return nsplit


# ------------------------------------------------------------------
# Full per-core program
# ------------------------------------------------------------------

def build_kernel(meta, nstreams=NSTREAM, taps=()):
    nc = bass.Bass()
    xin = nc.declare_dram_parameter("x", [NLOC, C, T, V, M], F32, isOutput=False)
    out = nc.declare_dram_parameter("out", [NLOC, NUM_CLASS], F32, isOutput=True)
    dram = {name: nc.declare_dram_parameter(f"w_{name}", list(shp), dt, isOutput=False)
            for name, (shp, dt) in meta.items()}

    with tile.TileContext(nc) as tc, ExitStack() as kctx:
        kb = KB(nc, tc)
        kb.load_weights(kctx, dram, meta)
        w = kb.w
        kb.pmm = kctx.enter_context(tc.tile_pool(name="pmm", bufs=3, space="PSUM"))
        kb.ptp = kctx.enter_context(tc.tile_pool(name="ptp", bufs=5, space="PSUM"))
        feat_pool = kctx.enter_context(tc.tile_pool(name="featp", bufs=1))
        fa = [feat_pool.tile([128, NLOC], DT, tag=f"feat{ct}", name=f"feat{ct}")
              for ct in range(2)]
        for ct in range(2):
            nc.vector.memset(fa[ct][:], 0.0)

        def tap(name, ap, rows, cols):
            if name in taps:
                d = nc.dram_tensor(f"tap_{name}", [rows, cols], ap.dtype,
                                   kind="ExternalOutput")
                nc.sync.dma_start(out=d[:], in_=ap)

        for si in range(nstreams):
            nl, mm = si // M, si % M
            with ExitStack() as sctx:
                keep = sctx.enter_context(tc.tile_pool(name=f"keep{si}", bufs=1))
                xsp = keep.tile([64, S50], DT, tag="xsp")
                f0p = keep.tile([64, S50], DT, tag="f0p")
                f1p = keep.tile([64, S50], DT, tag="f1p")
                f2p = keep.tile([64, S50], DT, tag="f2p")
                f3 = keep.tile([64, S50], DT, tag="f3")
                h = keep.tile([C, S300], F32, tag="h")

                # ---- P1: load + data_bn (fp32 stem) ----
                nc.sync.dma_start(out=h.rearrange("c (t v) -> c t v", v=V),
                                  in_=xin[nl, :, :, :, mm])
                h3 = h.rearrange("c (t v) -> c t v", v=V)
                for v in range(V):
                    nc.vector.tensor_scalar(h3[:, :, v], h3[:, :, v],
                                            w['dbn_s'][:, v:v + 1],
                                            w['dbn_b'][:, v:v + 1],
                                            op0=ALU.mult, op1=ALU.add)
                tap(f"hbn{si}", h[:], C, S300)

                with ExitStack() as wctx:
                    wk = wctx.enter_context(tc.tile_pool(name=f"work{si}", bufs=2))
                    PADS = S300 + 2 * V

                    # ---- P2: shift branch ----
                    s1 = wk.tile([64, PADS], DT, tag="big")
                    kb.conv(s1, [h[:]], [w['cs1_w'][:]], S300, bias=w['cs1_b'][:, 0:1])
                    tap(f"s1_{si}", s1[:, 0:S300], 64, S300)
                    pb = wk.tile([64, PADS], DT, tag="big")
                    kb.fill_pad(pb, 'ts', T)
                    kb.gcn(pb[:, V:V + S300], [s1[:, 0:S300]], 'g1', T,
                           res=s1[:, 0:S300])
                    tap(f"g1_{si}", pb[:, V:V + S300], 64, S300)
                    ts = wk.tile([128, PADS], DT, tag="big")
                    kb.tcn(ts, pb, 'ts', T, 1)
                    tap(f"ts_{si}", ts[:, 0:S300], 128, S300)
                    g2 = wk.tile([128, PADS], DT, tag="big")
                    kb.gcn(g2[:, 0:S300], [ts[:, 0:S300]], 'g2', T, res=ts[:, 0:S300])
                    tap(f"g2_{si}", g2[:, 0:S300], 128, S300)
                    kb.conv(h[0:2, :], [g2[:, 0:S300]], [w['cs2_w'][:]], S300,
                            bias=w['cs2_b'][:, 0:1], add_into=True)
                    tap(f"hs_{si}", h[:], C, S300)

                    # ---- P3: motion / pos / fusion ----
                    pbm = wk.tile([C, PADS], DT, tag="big")
                    kb.fill_pad(pbm, 'tm', T)
                    nc.vector.memset(pbm[:, V:2 * V], 0.0)
                    nc.vector.tensor_tensor(pbm[:, 2 * V:V + S300], h[:, V:S300],
                                            h[:, 0:S300 - V], op=ALU.subtract)
                    mo = wk.tile([64, S300], DT, tag="act64")
                    kb.tcn(mo, pbm, 'tm', T, 1)
                    pb1 = wk.tile([C, PADS], DT, tag="big")
                    kb.fill_pad(pb1, 'tp1', T)
                    nc.vector.tensor_copy(pb1[:, V:V + S300], h[:])
                    kb.tcn_pool(xsp, pb1, 'tp1', T, 6)
                    tap(f"xsp_{si}", xsp[:], 64, S50)
                    pb2 = wk.tile([C, PADS], DT, tag="big")
                    kb.fill_pad(pb2, 'tp2', T)
                    nc.vector.tensor_copy(pb2[:, V:V + S300], h[:])
                    p2t = wk.tile([64, S300], DT, tag="act64")
                    kb.tcn(p2t, pb2, 'tp2', T, 1)

                    pbt0 = wk.tile([64, PADS], DT, tag="big")
                    kb.fill_pad(pbt0, 'tl0', T)
                    kb.conv(pbt0, [p2t[:], mo[:]], [w['fu_w0'][:], w['fu_w1'][:]],
                            S300, bias=w['fu_b'][:, 0:1], dst_off=V)
                    tap(f"fu_{si}", pbt0[:, V:V + S300], 64, S300)

                    # ---- P4: tcn chain ----
                    pbt1 = wk.tile([64, PADS], DT, tag="big")
                    kb.fill_pad(pbt1, 'tl1', T)
                    kb.tcn(pbt1, pbt0, 'tl0', T, 1, dst_off=V)
                    pool_T_max(nc, f0p[:], pbt1[:, V:V + S300], T, 6)
                    pbt2 = wk.tile([64, PADS], DT, tag="big")
                    kb.fill_pad(pbt2, 'tl2', 150)
                    kb.tcn(pbt2, pbt1, 'tl1', T, 2, dst_off=V)
                    pool_T_max(nc, f1p[:], pbt2[:, V:V + S150], 150, 3)
                    pbt3 = wk.tile([64, PADS], DT, tag="big")
                    kb.fill_pad(pbt3, 'tl3', 150)
                    kb.tcn(pbt3, pbt2, 'tl2', 150, 1, dst_off=V)
                    pool_T_max(nc, f2p[:], pbt3[:, V:V + S150], 150, 3)
                    kb.tcn(f3, pbt3, 'tl3', 150, 3)
                    tap(f"f3_{si}", f3[:], 64, S50)

                # ---- P5: gcn_list + gather + head ----
                with ExitStack() as p5:
                    sp = p5.enter_context(tc.tile_pool(name=f"p5_{si}", bufs=1))
                    ga = sp.tile([64, S50], DT, tag="ga")
                    kb.gcn(ga, [xsp[:], f3[:]], 'gl0', 50, res='conv')
                    gb = sp.tile([64, S50], DT, tag="gb")
                    kb.gcn(gb, [ga[:], f2p[:]], 'gl1', 50, res='conv')
                    kb.gcn(ga, [gb[:], f1p[:]], 'gl2', 50, res='conv')
                    tap(f"gl2_{si}", ga[:], 64, S50)
                    gc0 = sp.tile([128, S50], DT, tag="gc0")
                    gc1 = sp.tile([128, S50], DT, tag="gc1")
                    kb.gcn([gc0, gc1], [ga[:], f0p[:]], 'gl3', 50, res='conv')
                    tap(f"gl3_{si}", gc0[:], 128, S50)

                    att = [sp.tile([128, S50], DT, tag=f"att{ct}", name=f"att{ct}")
                           for ct in range(2)]
                    kb.gather(att, [gc0, gc1], 'ga1', 50)
                    tap(f"ga1_{si}", att[0][:], 128, S50)
                    hv = [sp.tile([128, V], DT, tag=f"hv{ct}", name=f"hv{ct}")
                          for ct in range(2)]
                    for ct in range(2):
                        vv = att[ct].rearrange("c (t v) -> c v t", v=V)
                        nc.vector.tensor_reduce(hv[ct][:], vv, axis=AX.X, op=ALU.max)
                    av2 = [sp.tile([128, V], DT, tag=f"av2{ct}", name=f"av2{ct}")
                           for ct in range(2)]
                    kb.gather(av2, hv, 'ga2', 1)
                    tap(f"ga2_{si}", av2[0][:], 128, V)
                    for ct in range(2):
                        r = sp.tile([128, 1], DT, tag=f"r{ct}", name=f"r{ct}")
                        nc.vector.tensor_reduce(r[:], av2[ct][:], axis=AX.X, op=ALU.max)
                        if mm == 0:
                            nc.vector.tensor_copy(fa[ct][:, nl:nl + 1], r[:])
                        else:
                            nc.vector.tensor_tensor(fa[ct][:, nl:nl + 1],
                                                    fa[ct][:, nl:nl + 1], r[:],
                                                    op=ALU.max)

        # ---- final fcn ----
        ps = kb.pmm.tile([128, CHUNK], F32, tag="mm")
        kb.mm(ps[:NLOC, :NUM_CLASS], fa[0][:], w['fcn_w_p0'][:], start=True, stop=False)
        kb.mm(ps[:NLOC, :NUM_CLASS], fa[1][:], w['fcn_w_p1'][:], start=False, stop=True)
        osb = feat_pool.tile([NLOC, NUM_CLASS], F32, tag="osb")
        nc.vector.tensor_tensor(osb[:], ps[:NLOC, :NUM_CLASS], w['fcn_b'][:],
                                op=ALU.add)
        nc.sync.dma_start(out=out[:], in_=osb[:])

    return nc


# ------------------------------------------------------------------
# public entry
# ------------------------------------------------------------------

_CACHE = {}


def kernel(x, params):
    x = np.asarray(x, np.float32)
    W = prep_weights(params)
    meta = wmeta(W)
    if 'nc' not in _CACHE:
        nc = build_kernel(meta)
        split_multiwaits(nc)
        _CACHE['nc'] = nc
    nc = _CACHE['nc']
    in_maps = []
    for c in range(NCORES):
        im = {"x": np.ascontiguousarray(x[c * NLOC:(c + 1) * NLOC])}
        for k, v in W.items():
            im[f"w_{k}"] = v
        in_maps.append(im)
    res = run_bass_kernel_spmd(nc, in_maps, core_ids=list(range(NCORES)))
    return np.concatenate([res.results[c]["out"] for c in range(NCORES)], axis=0)
